# revision 17
# baseline (speedup 1.0000x reference)
"""Trainium2 Bass kernel for a batch-of-trees BinaryTreeLSTM (fp16 rewrite).

Contract: kernel(**inputs) takes the FULL inputs (B=128 trees, 1023-node
complete binary tree, dim 300) and returns the FULL output (root_c, root_h),
each [128, 300] float32.

Strategy
--------
- Data-parallel over trees: 16 trees per NeuronCore x 8 cores, no collectives.
- fp16 everywhere: GEMM operands (weights, x, h), gates, c/h states.  fp32
  PSUM accumulate + fp32 ACT internals keep the root error ~1e-3 (emulated),
  well under the 2e-2 gate.  fp16 runs 1 cycle/row on the PE at ANY free size
  (fp32r pays 4x below 256), halves LDWEIGHTS, DMA and SBUF vs fp32r.
- M-repacked gate units: the matmul cost model is (#units x #K-chunks) x N,
  independent of unit row-width, so the 1500 recurrent gate rows (i,o,u,fL,fR
  x 300) are packed into 12 units of <=128 rows (vs 15 naive) and the 900
  leaf gate rows into 8 units (vs 9).  Tail rows of several gates share units.
- Bias enters via a ones-row at partition 44 of the packed K-chunk 2 (the
  chunk that carries child-h features 256:300 of left/right at partitions
  0:44 / 64:108), so ACT applies pure sigmoid/tanh and pairs of units merge
  into single wide ACT instructions.
- All state is SBUF-resident (fp16 makes it fit); no DRAM round-trips for
  levels 1..3 anymore.  States stored deinterleaved ([even nodes | odd]) per
  feature chunk: h01/c01 [128, 2R] (chunks 0,1), h2p/c2p [128, R/2] with the
  44-row chunk-2 packed at partitions 0:44 (even) / 64:108 (odd).
- Phase A software-pipelines leaf blocks against the previous block's level-1
  GEMM so the PE never waits on the leaf elementwise chain.
"""

import os
import sys

for _p in ("/opt/trn_rl_repo",):
    if os.path.isdir(_p) and _p not in sys.path:
        sys.path.insert(0, _p)

import numpy as np
from contextlib import ExitStack

import concourse.bass as bass
import concourse.tile as tile
from concourse import bacc, mybir
from concourse.bass_utils import run_bass_kernel_spmd

# ---------------------------------------------------------------- constants
N_CORES = 8
B = 128
B_LOC = B // N_CORES          # 16 trees per core
N_LEAVES = 512
MEM = 300
XCOLS = N_LEAVES * B_LOC      # 8192 leaf columns per core
LB = 1024                     # leaf-block columns (64 leaves)
NF = 512                      # max moving free dim
R_LVL = {l: XCOLS >> l for l in range(1, 10)}   # level l column count

F16 = mybir.dt.float16
F32 = mybir.dt.float32
AF = mybir.ActivationFunctionType
SIG = AF.Sigmoid
TANH = AF.Tanh
MUL = mybir.AluOpType.mult
ADD = mybir.AluOpType.add

# Leaf M-units (8 units over Wiou cols [i 0:300 | o 300:600 | u 600:900]):
# unit -> list of (dst_row0, dst_row1, src_col0)
# (SBUF compute APs must start at partition 0 or 64, so tail gates sit at
# those offsets; rows 44:64 / 108:128 of tail units carry zero weights.)
LEAF_SLOTS = {
    0: [(0, 128, 0)], 1: [(0, 128, 128)],
    2: [(0, 128, 300)], 3: [(0, 128, 428)],
    4: [(0, 128, 600)], 5: [(0, 128, 728)],
    6: [(0, 44, 256), (64, 108, 556)],   # i2 @0 | o2 @64
    7: [(0, 44, 856)],                   # u2 @0
}
# Recurrent M-units (13 units over Wcat cols
# [i 0:300 | o 300:600 | u 600:900 | fL 900:1200 | fR 1200:1500]):
REC_SLOTS = {
    0: [(0, 128, 0)], 1: [(0, 128, 128)],
    2: [(0, 128, 300)], 3: [(0, 128, 428)],
    4: [(0, 128, 900)], 5: [(0, 128, 1028)],     # fL
    6: [(0, 128, 1200)], 7: [(0, 128, 1328)],    # fR
    8: [(0, 128, 600)], 9: [(0, 128, 728)],      # u
    10: [(0, 44, 256), (64, 108, 556)],          # i2 @0 | o2 @64
    11: [(0, 44, 856), (64, 108, 1156)],         # u2 @0 | fL2 @64
    12: [(64, 108, 1456)],                       # fR2 @64
}


# ---------------------------------------------------------------- host packing
def _pack_weights(Wfioux, b_fioux, Wiouh, Wfh):
    f4 = np.float32
    Wiou = np.asarray(Wfioux[:, 300:1200], f4)            # [300, 900]
    bleaf = np.asarray(b_fioux[300:1200], f4)             # [900]
    wleaf = np.zeros((3, 128, 8 * 128), f4)   # flattened to [128, 3*1024] below
    kch_l = [(0, 128), (128, 256), (256, 300)]
    for kc, (ra, rb) in enumerate(kch_l):
        for m, slots in LEAF_SLOTS.items():
            for (r0, r1, c0) in slots:
                wleaf[kc, 0: rb - ra, 128 * m + r0: 128 * m + r1] = \
                    Wiou[ra:rb, c0: c0 + (r1 - r0)]
    # bias via ones-row at partition 44 of K-chunk 2
    for m, slots in LEAF_SLOTS.items():
        for (r0, r1, c0) in slots:
            wleaf[2, 44, 128 * m + r0: 128 * m + r1] = bleaf[c0: c0 + (r1 - r0)]

    Wcat = np.concatenate([Wiouh, Wfh], axis=1).astype(f4)  # [600, 1500]
    bf = np.asarray(b_fioux[0:300], f4)
    bias_cat = np.concatenate(
        [b_fioux[300:600], b_fioux[600:900], b_fioux[900:1200], bf, bf]
    ).astype(f4)
    wrec = np.zeros((5, 128, 13 * 128), f4)
    # K-chunks: 0: hL[0:128], 1: hL[128:256], 2: packed hL[256:300]@0:44 +
    # ones@44 + hR[256:300]@64:108, 3: hR[0:128], 4: hR[128:256]
    kch_r = [(0, 128, 0), (128, 256, 0), None, (300, 428, 0), (428, 556, 0)]
    for kc, span in enumerate(kch_r):
        if span is None:
            continue
        ra, rb, _ = span
        for m, slots in REC_SLOTS.items():
            for (r0, r1, c0) in slots:
                wrec[kc, 0: rb - ra, 128 * m + r0: 128 * m + r1] = \
                    Wcat[ra:rb, c0: c0 + (r1 - r0)]
    for m, slots in REC_SLOTS.items():
        for (r0, r1, c0) in slots:
            wrec[2, 0:44, 128 * m + r0: 128 * m + r1] = \
                Wcat[256:300, c0: c0 + (r1 - r0)]
            wrec[2, 44, 128 * m + r0: 128 * m + r1] = bias_cat[c0: c0 + (r1 - r0)]
            wrec[2, 64:108, 128 * m + r0: 128 * m + r1] = \
                Wcat[556:600, c0: c0 + (r1 - r0)]
    wleaf_f = np.ascontiguousarray(
        wleaf.transpose(1, 0, 2).reshape(128, 3 * 1024))
    wrec_f = np.ascontiguousarray(
        wrec.transpose(1, 0, 2).reshape(128, 5 * 1664))
    return wleaf_f.astype(np.float16), wrec_f.astype(np.float16)


def _check_topology(left_idx, right_idx, leaf_mask):
    li = np.asarray(left_idx); ri = np.asarray(right_idx)
    prev = np.arange(N_LEAVES); nid = N_LEAVES
    ok = bool((np.asarray(leaf_mask)[:N_LEAVES] == 1).all())
    ok &= bool((np.asarray(leaf_mask)[N_LEAVES:] == 0).all())
    while len(prev) > 1:
        cur = []
        for k in range(0, len(prev), 2):
            ok &= bool(li[nid] == prev[k]) and bool(ri[nid] == prev[k + 1])
            cur.append(nid); nid += 1
        prev = np.asarray(cur)
    return ok


# ---------------------------------------------------------------- bass program
def _ev_od(ap, b=B_LOC):
    """Block-dense [p, X] (node-major, X = m*2*b) -> (even, odd) [p, m, b]."""
    r = ap.rearrange("p (m two b) -> p m two b", two=2, b=b)
    return r[:, :, 0, :], r[:, :, 1, :]


def _mb(ap, b=B_LOC):
    return ap.rearrange("p (m b) -> p m b", b=b)


def build_program():
    nc = bacc.Bacc("TRN2", target_bir_lowering=False, debug=False)

    xt_d = nc.dram_tensor("xt", [128, 2, XCOLS], F16, kind="ExternalInput").ap()
    x2t_d = nc.dram_tensor("x2t", [44, XCOLS], F16, kind="ExternalInput").ap()
    wleaf_d = nc.dram_tensor("wleaf", [128, 3 * 8 * 128], F16,
                             kind="ExternalInput").ap()
    wrec_d = nc.dram_tensor("wrec", [128, 5 * 13 * 128], F16,
                            kind="ExternalInput").ap()
    cons_d = nc.dram_tensor("cons", [84, 2 * LB], F16, kind="ExternalInput").ap()
    out_d = nc.dram_tensor("out", [2, MEM, B_LOC], F16, kind="ExternalOutput").ap()

    with ExitStack() as ctx:
        tc = ctx.enter_context(tile.TileContext(nc))
        _build(ctx, tc, xt_d, x2t_d, wleaf_d, wrec_d, cons_d, out_d)

    nc.compile()
    return nc


def _build(ctx, tc, xt_d, x2t_d, wleaf_d, wrec_d, cons_d, out_d):
    nc = tc.nc

    wpool = ctx.enter_context(tc.tile_pool(name="wpool", bufs=1))
    state_pool = ctx.enter_context(tc.tile_pool(name="state", bufs=1))

    # ---- weights resident in SBUF (leaf weights first: needed immediately)
    wleaf_t = wpool.tile([128, 3 * 8 * 128], F16, name="wleaf")
    nc.sync.dma_start(wleaf_t[:], wleaf_d[:])
    wleaf_sb = [wleaf_t[:, k * 1024: (k + 1) * 1024] for k in range(3)]
    wrec_t = wpool.tile([128, 5 * 13 * 128], F16, name="wrec")
    wrec_sb = [wrec_t[:, k * 1664: (k + 1) * 1664] for k in range(5)]

    # ---- persistent SBUF state for levels 1..8
    ST = {}
    for lvl in range(1, 9):
        R = R_LVL[lvl]
        h01 = state_pool.tile([128, 2 * R], F16, name=f"h01_{lvl}")
        h2p = state_pool.tile([128, R // 2], F16, name=f"h2p_{lvl}")
        c01 = state_pool.tile([128, 2 * R], F16, name=f"c01_{lvl}")
        c2p = state_pool.tile([128, R], F16, name=f"c2p_{lvl}")
        ST[lvl] = dict(h01=h01, h2p=h2p, c01=c01, c2p=c2p, R=R)

    # persistent double-buffered leaf tiles that carry constant rows
    x2_t = []
    lh2p_t = []
    for i in range(2):
        t = state_pool.tile([128, LB], F16, name=f"x2_{i}")
        x2_t.append(t)
        t = state_pool.tile([128, LB // 2], F16, name=f"lh2p_{i}")
        lh2p_t.append(t)

    def _pad_dmas():
        """Pad/ones constants for tiles not needed in the first block; emitted
        after the block-0 x DMAs so compute starts immediately."""
        for i in range(2):
            t = lh2p_t[i]
            nc.sync.dma_start(t[44:64, :], cons_d[0:20, : LB // 2])
            nc.sync.dma_start(t[108:128, :], cons_d[1:21, : LB // 2])
        for lvl in range(1, 9):
            R = R_LVL[lvl]
            h2p = ST[lvl]["h2p"]
            nc.sync.dma_start(h2p[44:64, :], cons_d[0:20, : R // 2])
            nc.sync.dma_start(h2p[108:128, :], cons_d[1:21, : R // 2])

    # ---- pools
    xpool = ctx.enter_context(tc.tile_pool(name="xpool", bufs=2))
    glpool = ctx.enter_context(tc.tile_pool(name="gl", bufs=2))
    lpool = ctx.enter_context(tc.tile_pool(name="lpool", bufs=2))
    gpool = ctx.enter_context(tc.tile_pool(name="g", bufs=2))
    pspool = ctx.enter_context(tc.tile_pool(name="ps", bufs=4, space="PSUM"))
    tmp1 = ctx.enter_context(tc.tile_pool(name="tmp1", bufs=1))
    tmp2 = ctx.enter_context(tc.tile_pool(name="tmp2", bufs=2))
    tmp3 = ctx.enter_context(tc.tile_pool(name="tmp3", bufs=1))
    opool = ctx.enter_context(tc.tile_pool(name="o", bufs=1))

    # ================================================================ helpers
    def leaf_gemm(xk, s, Gl):
        """Leaf gates for sub-chunk s (512 cols): 4 psum pairs, 6 ACTs."""
        n0 = s * NF
        for pi in range(4):
            ua, ub = 2 * pi, 2 * pi + 1
            ps = pspool.tile([128, 2 * NF], F32, tag="ps", name=f"psl{pi}")
            for j, u in enumerate((ua, ub)):
                rows = (128, 128, 128, 128, 128, 128, 108, 44)[u]
                off = j * NF
                for kc in range(3):
                    nc.tensor.matmul(
                        ps[0:rows, off: off + NF],
                        wleaf_sb[kc][:, 128 * u: 128 * u + rows],
                        xk[kc][:, n0: n0 + NF],
                        start=(kc == 0), stop=(kc == 2))
            if pi < 3:
                func = SIG if pi < 2 else TANH
                nc.scalar.activation(Gl[:, 2 * pi * NF: (2 * pi + 2) * NF],
                                     ps[:, :], func)
            else:
                nc.scalar.activation(Gl[0:108, 6 * NF: 7 * NF],
                                     ps[0:108, 0:NF], SIG)
                nc.scalar.activation(Gl[0:44, 7 * NF: 8 * NF],
                                     ps[0:44, NF: NF + NF], TANH)

    def rec_gemm(rhs_k, PB, G):
        """Recurrent gates for one block of PB cols: 6 psum pairs + 1 single."""
        USPAN = ((0, 128),) * 10 + ((0, 108), (0, 108), (64, 108))
        for pi in range(7):
            units = (2 * pi, 2 * pi + 1) if pi < 6 else (12,)
            ps = pspool.tile([128, 2 * NF], F32, tag="ps", name=f"psr{pi}")
            for j, u in enumerate(units):
                r0, r1 = USPAN[u]
                off = j * PB
                for kc in range(5):
                    nc.tensor.matmul(
                        ps[r0:r1, off: off + PB],
                        wrec_sb[kc][:, 128 * u + r0: 128 * u + r1],
                        rhs_k[kc],
                        start=(kc == 0), stop=(kc == 4),
                        tile_position=(0, 64) if r0 else None)
            if pi < 5:
                func = SIG if pi < 4 else TANH
                nc.scalar.activation(G[:, 2 * pi * PB: (2 * pi + 2) * PB],
                                     ps[:, 0: 2 * PB], func)
            elif pi == 5:
                # T10 = [i2@0 | o2@64] all sigmoid; T11 = [u2@0 | fL2@64]
                nc.scalar.activation(G[0:108, 10 * PB: 11 * PB],
                                     ps[0:108, 0:PB], SIG)
                nc.scalar.activation(G[0:44, 11 * PB: 12 * PB],
                                     ps[0:44, PB: 2 * PB], TANH)
                nc.scalar.activation(G[64:108, 11 * PB: 12 * PB],
                                     ps[64:108, PB: 2 * PB], SIG)
            else:
                nc.scalar.activation(G[64:108, 12 * PB: 13 * PB],
                                     ps[64:108, 0:PB], SIG)

    def st_sl(t, R, ch, eo, q0, w):
        off = ch * R + eo * (R // 2) + q0
        return t[:, off: off + w]

    def _eo_mb(ap):
        """Block-dense [p, X] node-major -> [p, eo, m, b] (dims reordered)."""
        return ap.rearrange("p (m eo b) -> p eo m b", eo=2, b=B_LOC)

    def st_eo(t, R, ch, q0, w):
        """State [p, 2R] -> [p, eo, m, b] covering E+O slices [q0:q0+w] of ch."""
        v = t.rearrange("p (ch eo m b) -> p ch eo m b", ch=2, eo=2, b=B_LOC)
        return v[:, ch, :, q0 // B_LOC: (q0 + w) // B_LOC, :]

    def rec_ew(G, PB, CL01, CR01, CL2, CR2, dst, p0):
        """Elementwise for a recurrent block. G gates [128, 13*PB].
        CL01/CR01: [p, 2, PB] child-c chunk-0/1 views; CL2/CR2 [44, PB].
        dst: ST[lvl] dict, or ('root', oc01, oc2, oh01, oh2) for level 9."""
        N = PB
        gi2 = G[0:44, 10 * N: 11 * N]
        go = [G[:, 2 * N: 3 * N], G[:, 3 * N: 4 * N], G[64:108, 10 * N: 11 * N]]
        gfL2 = G[64:108, 11 * N: 12 * N]
        gfR2 = G[64:108, 12 * N: 13 * N]
        gu2 = G[0:44, 11 * N: 12 * N]
        ch2 = lambda ap: ap.rearrange("p (ch q) -> p ch q", ch=2)

        t1 = tmp1.tile([128, 2 * NF], F16, tag="t1", name="t1")
        t2 = tmp1.tile([128, 2 * NF], F16, tag="t2", name="t2")
        fc = tmp1.tile([128, 2 * NF], F16, tag="fc", name="fc")
        iu = tmp1.tile([128, 2 * NF], F16, tag="iu", name="iu")
        t1_2 = tmp1.tile([44, NF], F16, tag="t1_2", name="t1_2")
        t2_2 = tmp1.tile([44, NF], F16, tag="t2_2", name="t2_2")
        fc2 = tmp1.tile([44, NF], F16, tag="fc2", name="fc2")
        iu2 = tmp1.tile([44, NF], F16, tag="iu2", name="iu2")

        nc.vector.tensor_tensor(ch2(t1[:, : 2 * N]), ch2(G[:, 4 * N: 6 * N]),
                                CL01, MUL)
        nc.vector.tensor_tensor(ch2(t2[:, : 2 * N]), ch2(G[:, 6 * N: 8 * N]),
                                CR01, MUL)
        nc.vector.tensor_tensor(t1_2[:, :N], gfL2, CL2, MUL)
        nc.vector.tensor_tensor(t2_2[:, :N], gfR2, CR2, MUL)
        nc.vector.tensor_tensor(fc[:, : 2 * N], t1[:, : 2 * N], t2[:, : 2 * N],
                                ADD)
        nc.vector.tensor_tensor(fc2[:, :N], t1_2[:, :N], t2_2[:, :N], ADD)
        nc.vector.tensor_tensor(iu[:, : 2 * N], G[:, 0: 2 * N],
                                G[:, 8 * N: 10 * N], MUL)
        nc.vector.tensor_tensor(iu2[:, :N], gi2, gu2, MUL)

        if isinstance(dst, tuple) and dst[0] == "root":
            _, oc01, oc2, oh01, oh2 = dst
            nc.vector.tensor_tensor(oc01[:, : 2 * N], iu[:, : 2 * N],
                                    fc[:, : 2 * N], ADD)
            nc.vector.tensor_tensor(oc2[:, :N], iu2[:, :N], fc2[:, :N], ADD)
            th = tmp2.tile([128, 2 * NF], F16, tag="th", name="th")
            th2 = tmp3.tile([128, NF], F16, tag="th2", name="th2")
            nc.scalar.activation(th[:, : 2 * N], oc01[:, : 2 * N], TANH)
            nc.scalar.activation(th2[64:108, :N], oc2[:, :N], TANH)
            nc.vector.tensor_tensor(oh01[:, : 2 * N], G[:, 2 * N: 4 * N],
                                    th[:, : 2 * N], MUL)
            nc.vector.tensor_tensor(oh2[:, :N], go[2], th2[64:108, :N], MUL)
            return

        st = dst
        R = st["R"]
        q0, hw = p0 // 2, PB // 2
        # c writes (deinterleave into state, E+O fused), tanh, h writes
        for ch in range(2):
            nc.vector.tensor_tensor(st_eo(st["c01"], R, ch, q0, hw),
                                    _eo_mb(iu[:, ch * N: (ch + 1) * N]),
                                    _eo_mb(fc[:, ch * N: (ch + 1) * N]), ADD)
        c2o = st["c2p"].rearrange("p (eo m b) -> p eo m b", eo=2, b=B_LOC)
        nc.vector.tensor_tensor(
            c2o[64:108, :, q0 // B_LOC: (q0 + hw) // B_LOC, :],
            _eo_mb(iu2[:, :N]), _eo_mb(fc2[:, :N]), ADD)

        # th layout: [ch0E | ch0O | ch1E | ch1O], each hw wide
        th = tmp2.tile([128, 2 * NF], F16, tag="th", name="th")
        th2 = tmp3.tile([128, NF], F16, tag="th2", name="th2")
        c4 = st["c01"].rearrange("p (ch eo q) -> p ch eo q", ch=2, eo=2)
        tho = th[:, : 2 * N].rearrange("p (ch eo q) -> p ch eo q", ch=2, eo=2)
        nc.scalar.activation(tho, c4[:, :, :, q0: q0 + hw], TANH)
        c2v = st["c2p"].rearrange("p (eo q) -> p eo q", eo=2)
        nc.scalar.activation(
            th2[64:108, :N].rearrange("p (eo q) -> p eo q", eo=2),
            c2v[64:108, :, q0: q0 + hw], TANH)

        for ch in range(2):
            nc.vector.tensor_tensor(
                st_eo(st["h01"], R, ch, q0, hw), _eo_mb(go[ch]),
                th[:, ch * N: (ch + 1) * N].rearrange(
                    "p (eo m b) -> p eo m b", eo=2, b=B_LOC), MUL)
        oe, oo = _ev_od(go[2])
        nc.vector.tensor_tensor(_mb(st["h2p"][0:44, q0: q0 + hw]), oe,
                                _mb(th2[64:108, 0:hw]), MUL)
        nc.vector.tensor_tensor(_mb(st["h2p"][64:108, q0: q0 + hw]), oo,
                                _mb(th2[64:108, hw:N]), MUL)

    def leaf_ew(Gl, s, lh01, lh2p, lc01, lc2p):
        """Leaf elementwise for sub-chunk s (512 cols): c = i*u, h = o*tanh(c).
        Writes deinterleaved into the LB-wide block-local leaf tiles."""
        N = NF
        q0, hw = s * (NF // 2), NF // 2
        go = [Gl[:, 2 * N: 3 * N], Gl[:, 3 * N: 4 * N], Gl[64:108, 6 * N: 7 * N]]

        for ch in range(2):
            nc.vector.tensor_tensor(st_eo(lc01, LB, ch, q0, hw),
                                    _eo_mb(Gl[:, ch * N: (ch + 1) * N]),
                                    _eo_mb(Gl[:, (4 + ch) * N: (5 + ch) * N]),
                                    MUL)
        c2o = lc2p.rearrange("p (eo m b) -> p eo m b", eo=2, b=B_LOC)
        nc.vector.tensor_tensor(
            c2o[64:108, :, q0 // B_LOC: (q0 + hw) // B_LOC, :],
            _eo_mb(Gl[0:44, 6 * N: 7 * N]), _eo_mb(Gl[0:44, 7 * N: 8 * N]), MUL)

        th = tmp3.tile([128, 2 * NF], F16, tag="thl", name="lth")
        th2 = tmp3.tile([128, NF], F16, tag="th2l", name="lth2")
        c4 = lc01.rearrange("p (ch eo q) -> p ch eo q", ch=2, eo=2)
        tho = th[:, : 2 * N].rearrange("p (ch eo q) -> p ch eo q", ch=2, eo=2)
        nc.scalar.activation(tho, c4[:, :, :, q0: q0 + hw], TANH)
        c2v = lc2p.rearrange("p (eo q) -> p eo q", eo=2)
        nc.scalar.activation(
            th2[64:108, :N].rearrange("p (eo q) -> p eo q", eo=2),
            c2v[64:108, :, q0: q0 + hw], TANH)

        for ch in range(2):
            nc.vector.tensor_tensor(
                st_eo(lh01, LB, ch, q0, hw), _eo_mb(go[ch]),
                th[:, ch * N: (ch + 1) * N].rearrange(
                    "p (eo m b) -> p eo m b", eo=2, b=B_LOC), MUL)
        oe, oo = _ev_od(go[2])
        nc.vector.tensor_tensor(_mb(lh2p[0:44, q0: q0 + hw]), oe,
                                _mb(th2[64:108, 0:hw]), MUL)
        nc.vector.tensor_tensor(_mb(lh2p[64:108, q0: q0 + hw]), oo,
                                _mb(th2[64:108, hw:N]), MUL)

    # ================================================================ phase A
    # leaves + level-1, software-pipelined: L1 GEMM of block b-1 is emitted
    # after the leaf GEMMs of block b so the PE never waits on leaf DVE.
    n_blk = XCOLS // LB                       # 8 blocks
    pend = None                               # (lh01, lh2p, lc01, lc2p, blk)

    def l1_block(lh01, lh2p, lc01, lc2p, blk):
        rhs_k = [st_sl(lh01, LB, 0, 0, 0, NF), st_sl(lh01, LB, 1, 0, 0, NF),
                 lh2p[:, :],
                 st_sl(lh01, LB, 0, 1, 0, NF), st_sl(lh01, LB, 1, 1, 0, NF)]
        G = gpool.tile([128, 13 * NF], F16, tag="G", name="G1")
        rec_gemm(rhs_k, NF, G)
        cv = lc01.rearrange("p (ch x) -> p ch x", ch=2)
        rec_ew(G, NF, cv[:, :, 0: LB // 2], cv[:, :, LB // 2: LB],
               lc2p[64:108, 0: LB // 2], lc2p[64:108, LB // 2: LB],
               ST[1], blk * NF)

    for blk in range(n_blk):
        c0 = blk * LB
        x01 = xpool.tile([128, 2 * LB], F16, tag="x01", name="x01")
        x2 = x2_t[blk % 2]
        nc.sync.dma_start(x01.rearrange("p (two b) -> p two b", two=2),
                          xt_d[:, :, c0: c0 + LB])
        nc.sync.dma_start(x2[0:44, :], x2t_d[:, c0: c0 + LB])
        if blk == 0:
            nc.sync.dma_start(x2[44:128, :], cons_d[0:84, :LB])
            nc.sync.dma_start(wrec_t[:], wrec_d[:])
            nc.sync.dma_start(x2_t[1][44:128, :], cons_d[0:84, :LB])
        elif blk == 1:
            _pad_dmas()
        xk = [x01[:, 0:LB], x01[:, LB: 2 * LB], x2]

        lh01 = lpool.tile([128, 2 * LB], F16, tag="lh01", name="lh01")
        lh2p = lh2p_t[blk % 2]
        lc01 = lpool.tile([128, 2 * LB], F16, tag="lc01", name="lc01")
        lc2p = lpool.tile([128, LB], F16, tag="lc2p", name="lc2p")

        Gls = []
        for s in range(2):
            Gl = glpool.tile([128, 8 * NF], F16, tag="Gl", name="Gl")
            leaf_gemm(xk, s, Gl)
            Gls.append(Gl)
        if pend is not None:
            l1_block(*pend)
        for s in range(2):
            leaf_ew(Gls[s], s, lh01, lh2p, lc01, lc2p)
        pend = (lh01, lh2p, lc01, lc2p, blk)
    l1_block(*pend)

    # ================================================================ phase B
    for lvl in range(2, 10):
        R = R_LVL[lvl]
        Rp = R_LVL[lvl - 1]
        PB = min(NF, R)
        prev = ST[lvl - 1]
        for p0 in range(0, R, PB):
            rhs_k = [st_sl(prev["h01"], Rp, 0, 0, p0, PB),
                     st_sl(prev["h01"], Rp, 1, 0, p0, PB),
                     prev["h2p"][:, p0: p0 + PB],
                     st_sl(prev["h01"], Rp, 0, 1, p0, PB),
                     st_sl(prev["h01"], Rp, 1, 1, p0, PB)]
            G = gpool.tile([128, 13 * NF], F16, tag="G", name=f"G{lvl}")
            rec_gemm(rhs_k, PB, G[:, : 13 * PB])
            cv = prev["c01"].rearrange("p (ch x) -> p ch x", ch=2)
            CL01 = cv[:, :, p0: p0 + PB]
            CR01 = cv[:, :, Rp // 2 + p0: Rp // 2 + p0 + PB]
            CL2 = prev["c2p"][64:108, p0: p0 + PB]
            CR2 = prev["c2p"][64:108, Rp // 2 + p0: Rp // 2 + p0 + PB]
            if lvl < 9:
                rec_ew(G[:, : 13 * PB], PB, CL01, CR01, CL2, CR2, ST[lvl], p0)
            else:
                oc01 = opool.tile([128, 2 * B_LOC], F16, name="oc01")
                oc2 = opool.tile([44, B_LOC], F16, name="oc2")
                oh01 = opool.tile([128, 2 * B_LOC], F16, name="oh01")
                oh2 = opool.tile([44, B_LOC], F16, name="oh2")
                rec_ew(G[:, : 13 * PB], PB, CL01, CR01, CL2, CR2,
                       ("root", oc01, oc2, oh01, oh2), p0)
                nc.sync.dma_start(out_d[0, 0:128, :], oc01[:, 0:B_LOC])
                nc.sync.dma_start(out_d[0, 128:256, :], oc01[:, B_LOC: 2 * B_LOC])
                nc.sync.dma_start(out_d[0, 256:300, :], oc2[:, :])
                nc.sync.dma_start(out_d[1, 0:128, :], oh01[:, 0:B_LOC])
                nc.sync.dma_start(out_d[1, 128:256, :], oh01[:, B_LOC: 2 * B_LOC])
                nc.sync.dma_start(out_d[1, 256:300, :], oh2[:, :])


# ---------------------------------------------------------------- runner
_CACHE = {}


def _get_program():
    if "nc" not in _CACHE:
        _CACHE["nc"] = build_program()
    return _CACHE["nc"]


def _host_inputs(inputs, Wfioux, b_fioux, Wiouh, Wfh):
    wleaf, wrec = _pack_weights(
        np.asarray(Wfioux, np.float32), np.asarray(b_fioux, np.float32),
        np.asarray(Wiouh, np.float32), np.asarray(Wfh, np.float32))
    cons = np.zeros((84, 2 * LB), np.float16)
    cons[0, :] = 1.0
    in_maps = []
    for core in range(N_CORES):
        x = np.asarray(inputs[core * B_LOC:(core + 1) * B_LOC, :N_LEAVES, :],
                       np.float32)
        xt_full = x.transpose(2, 1, 0).reshape(MEM, XCOLS).astype(np.float16)
        xt = np.ascontiguousarray(xt_full[0:256].reshape(2, 128, XCOLS)
                                  .transpose(1, 0, 2))
        x2t = np.ascontiguousarray(xt_full[256:300])
        in_maps.append({"xt": xt, "x2t": x2t, "wleaf": wleaf, "wrec": wrec,
                        "cons": cons})
    return in_maps


def kernel(inputs, Wfioux, b_fioux, Wiouh, Wfh, left_idx, right_idx, leaf_mask,
           _trace=False, _trace_dir=None):
    inputs = np.asarray(inputs, np.float32)
    assert _check_topology(left_idx, right_idx, leaf_mask), \
        "tree topology does not match the expected complete binary tree"

    in_maps = _host_inputs(inputs, Wfioux, b_fioux, Wiouh, Wfh)
    nc = _get_program()
    res = run_bass_kernel_spmd(nc, in_maps, list(range(N_CORES)),
                               trace=_trace, tmpdir=_trace_dir)

    root_c = np.zeros((B, MEM), np.float32)
    root_h = np.zeros((B, MEM), np.float32)
    for core in range(N_CORES):
        out = np.asarray(res.results[core]["out"], np.float32)  # [2, 300, 16]
        root_c[core * B_LOC:(core + 1) * B_LOC] = out[0].T
        root_h[core * B_LOC:(core + 1) * B_LOC] = out[1].T
    _CACHE["last_results"] = res
    return root_c, root_h


# revision 18
# speedup vs baseline: 1.1975x; 1.1975x over previous
"""Trainium2 Bass kernel for a batch-of-trees BinaryTreeLSTM (fp16 rewrite).

Contract: kernel(**inputs) takes the FULL inputs (B=128 trees, 1023-node
complete binary tree, dim 300) and returns the FULL output (root_c, root_h),
each [128, 300] float32.

Strategy
--------
- Data-parallel over trees: 16 trees per NeuronCore x 8 cores, no collectives.
- fp16 everywhere: GEMM operands (weights, x, h), gates, c/h states.  fp32
  PSUM accumulate + fp32 ACT internals keep the root error ~1.3e-3, well
  under the 2e-2 gate.  fp16 runs 1 cycle/row on the PE at ANY free size
  (fp32r pays 4x below 256 cols), halves LDWEIGHTS, DMA and SBUF vs fp32r.
- M-repacked gate units: matmul cost is (#units x #K-chunks) x N, independent
  of unit row-width, so the 1500 recurrent gate rows (i,o,u,fL,fR x 300) are
  packed into 13 units (vs 15 naive; 44-row gate tails sit at partition
  offsets 0/64 to satisfy the SBUF base-partition rule) and the 900 leaf gate
  rows into 8 units.
- Bias enters via a ones-row at partition 44 of the packed K-chunk 2 (which
  carries child-h features 256:300 of left/right at partitions 0:44/64:108),
  so ACT applies pure sigmoid/tanh and unit pairs merge into wide ACTs.
- All state is SBUF-resident (fp16 makes it fit); states deinterleaved
  ([even | odd] nodes) per feature chunk: h01/c01 [128, 2R], h2p/c2p
  [128, R/2] with chunk 2 packed into partition bands.
- Phase A software-pipelines leaf blocks against the previous block's level-1
  GEMM so the PE never waits on the leaf elementwise chain.
- DMAs are batched (single wleaf/wrec transfers, 2-DMA x blocks, pad DMAs
  deferred past block 0) so the first matmul issues ~13us after start.
"""

import os
import sys

for _p in ("/opt/trn_rl_repo",):
    if os.path.isdir(_p) and _p not in sys.path:
        sys.path.insert(0, _p)

import numpy as np
from contextlib import ExitStack

import concourse.bass as bass
import concourse.tile as tile
from concourse import bacc, mybir
from concourse.bass_utils import run_bass_kernel_spmd

# ---------------------------------------------------------------- constants
N_CORES = 8
B = 128
B_LOC = B // N_CORES          # 16 trees per core
N_LEAVES = 512
MEM = 300
XCOLS = N_LEAVES * B_LOC      # 8192 leaf columns per core
LB = 1024                     # leaf-block columns (64 leaves)
NF = 512                      # max moving free dim
R_LVL = {l: XCOLS >> l for l in range(1, 10)}   # level l column count

F16 = mybir.dt.float16
F32 = mybir.dt.float32
AF = mybir.ActivationFunctionType
SIG = AF.Sigmoid
TANH = AF.Tanh
MUL = mybir.AluOpType.mult
ADD = mybir.AluOpType.add

# (SBUF compute APs must start at partition 0 or 64, so tail gates sit at
# those offsets; rows 44:64 / 108:128 of tail units carry zero weights.)
LEAF_SLOTS = {
    0: [(0, 128, 0)], 1: [(0, 128, 128)],
    2: [(0, 128, 300)], 3: [(0, 128, 428)],
    4: [(0, 128, 600)], 5: [(0, 128, 728)],
    6: [(0, 44, 256), (64, 108, 556)],   # i2 @0 | o2 @64
    7: [(0, 44, 856)],                   # u2 @0
}
# Recurrent M-units (13 units over Wcat cols
# [i 0:300 | o 300:600 | u 600:900 | fL 900:1200 | fR 1200:1500]):
REC_SLOTS = {
    0: [(0, 128, 0)], 1: [(0, 128, 128)],
    2: [(0, 128, 300)], 3: [(0, 128, 428)],
    4: [(0, 128, 900)], 5: [(0, 128, 1028)],     # fL
    6: [(0, 128, 1200)], 7: [(0, 128, 1328)],    # fR
    8: [(0, 128, 600)], 9: [(0, 128, 728)],      # u
    10: [(0, 44, 256), (64, 108, 556)],          # i2 @0 | o2 @64
    11: [(0, 44, 856), (64, 108, 1156)],         # u2 @0 | fL2 @64
    12: [(0, 44, 1456)],                         # fR2 @0
}


# ---------------------------------------------------------------- host packing
def _pack_weights(Wfioux, b_fioux, Wiouh, Wfh):
    f4 = np.float32
    Wiou = np.asarray(Wfioux[:, 300:1200], f4)            # [300, 900]
    bleaf = np.asarray(b_fioux[300:1200], f4)             # [900]
    wleaf = np.zeros((3, 128, 8 * 128), f4)
    kch_l = [(0, 128), (128, 256), (256, 300)]
    for kc, (ra, rb) in enumerate(kch_l):
        for m, slots in LEAF_SLOTS.items():
            for (r0, r1, c0) in slots:
                wleaf[kc, 0: rb - ra, 128 * m + r0: 128 * m + r1] = \
                    Wiou[ra:rb, c0: c0 + (r1 - r0)]
    # bias via ones-row at partition 44 of K-chunk 2
    for m, slots in LEAF_SLOTS.items():
        for (r0, r1, c0) in slots:
            wleaf[2, 44, 128 * m + r0: 128 * m + r1] = bleaf[c0: c0 + (r1 - r0)]

    Wcat = np.concatenate([Wiouh, Wfh], axis=1).astype(f4)  # [600, 1500]
    bf = np.asarray(b_fioux[0:300], f4)
    bias_cat = np.concatenate(
        [b_fioux[300:600], b_fioux[600:900], b_fioux[900:1200], bf, bf]
    ).astype(f4)
    wrec = np.zeros((5, 128, 13 * 128), f4)
    # K-chunks: 0: hL[0:128], 1: hL[128:256], 2: packed hL[256:300]@0:44 +
    # ones@44 + hR[256:300]@64:108, 3: hR[0:128], 4: hR[128:256]
    kch_r = [(0, 128), (128, 256), None, (300, 428), (428, 556)]
    for kc, span in enumerate(kch_r):
        if span is None:
            continue
        ra, rb = span
        for m, slots in REC_SLOTS.items():
            for (r0, r1, c0) in slots:
                wrec[kc, 0: rb - ra, 128 * m + r0: 128 * m + r1] = \
                    Wcat[ra:rb, c0: c0 + (r1 - r0)]
    for m, slots in REC_SLOTS.items():
        for (r0, r1, c0) in slots:
            wrec[2, 0:44, 128 * m + r0: 128 * m + r1] = \
                Wcat[256:300, c0: c0 + (r1 - r0)]
            wrec[2, 44, 128 * m + r0: 128 * m + r1] = bias_cat[c0: c0 + (r1 - r0)]
            wrec[2, 64:108, 128 * m + r0: 128 * m + r1] = \
                Wcat[556:600, c0: c0 + (r1 - r0)]
    wleaf_f = np.ascontiguousarray(
        wleaf.transpose(1, 0, 2).reshape(128, 3 * 1024))
    wrec_f = np.ascontiguousarray(
        wrec.transpose(1, 0, 2).reshape(128, 5 * 1664))
    return wleaf_f.astype(np.float16), wrec_f.astype(np.float16)


def _check_topology(left_idx, right_idx, leaf_mask):
    li = np.asarray(left_idx); ri = np.asarray(right_idx)
    prev = np.arange(N_LEAVES); nid = N_LEAVES
    ok = bool((np.asarray(leaf_mask)[:N_LEAVES] == 1).all())
    ok &= bool((np.asarray(leaf_mask)[N_LEAVES:] == 0).all())
    while len(prev) > 1:
        cur = []
        for k in range(0, len(prev), 2):
            ok &= bool(li[nid] == prev[k]) and bool(ri[nid] == prev[k + 1])
            cur.append(nid); nid += 1
        prev = np.asarray(cur)
    return ok


# ---------------------------------------------------------------- bass program
def _ev_od(ap, b=B_LOC):
    """Block-dense [p, X] (node-major, X = m*2*b) -> (even, odd) [p, m, b]."""
    r = ap.rearrange("p (m two b) -> p m two b", two=2, b=b)
    return r[:, :, 0, :], r[:, :, 1, :]


def _mb(ap, b=B_LOC):
    return ap.rearrange("p (m b) -> p m b", b=b)


def build_program():
    nc = bacc.Bacc("TRN2", target_bir_lowering=False, debug=False)

    xt_d = nc.dram_tensor("xt", [128, 2, XCOLS], F16, kind="ExternalInput").ap()
    x2t_d = nc.dram_tensor("x2t", [44, XCOLS], F16, kind="ExternalInput").ap()
    wleaf_d = nc.dram_tensor("wleaf", [128, 3 * 8 * 128], F16,
                             kind="ExternalInput").ap()
    wrec_d = nc.dram_tensor("wrec", [128, 5 * 13 * 128], F16,
                            kind="ExternalInput").ap()
    cons_d = nc.dram_tensor("cons", [84, 2 * LB], F16, kind="ExternalInput").ap()
    out_d = nc.dram_tensor("out", [2, MEM, B_LOC], F16, kind="ExternalOutput").ap()

    with ExitStack() as ctx:
        tc = ctx.enter_context(tile.TileContext(nc))
        _build(ctx, tc, xt_d, x2t_d, wleaf_d, wrec_d, cons_d, out_d)

    nc.compile()
    return nc


def _build(ctx, tc, xt_d, x2t_d, wleaf_d, wrec_d, cons_d, out_d):
    nc = tc.nc

    wpool = ctx.enter_context(tc.tile_pool(name="wpool", bufs=1))
    state_pool = ctx.enter_context(tc.tile_pool(name="state", bufs=1))

    # ---- weights resident in SBUF (leaf weights first: needed immediately)
    wleaf_t = wpool.tile([128, 3 * 8 * 128], F16, name="wleaf")
    nc.sync.dma_start(wleaf_t[:], wleaf_d[:])
    wleaf_sb = [wleaf_t[:, k * 1024: (k + 1) * 1024] for k in range(3)]
    wrec_t = wpool.tile([128, 5 * 13 * 128], F16, name="wrec")
    wrec_sb = [wrec_t[:, k * 1664: (k + 1) * 1664] for k in range(5)]

    # ---- persistent SBUF state for levels 1..8
    # h01/c01 [128, 2R]: chunks 0,1, each [E | O].  h2p [128, R/2]: chunk-2
    # packed E@0:44 / O@64:108 (+ ones row 44 for the GEMM bias).
    # c2p [128, R/2]: chunk-2 E@64:108 / O@0:44 (so fL2@64*CL2 and fR2@0*CR2
    # have matching input base partitions).
    ST = {}
    for lvl in range(1, 9):
        R = R_LVL[lvl]
        h01 = state_pool.tile([128, 2 * R], F16, name=f"h01_{lvl}")
        h2p = state_pool.tile([128, R // 2], F16, name=f"h2p_{lvl}")
        c01 = state_pool.tile([128, 2 * R], F16, name=f"c01_{lvl}")
        c2p = state_pool.tile([128, R // 2], F16, name=f"c2p_{lvl}")
        ST[lvl] = dict(h01=h01, h2p=h2p, c01=c01, c2p=c2p, R=R)

    # persistent double-buffered leaf tiles that carry constant rows
    x2_t = []
    lh2p_t = []
    for i in range(2):
        x2_t.append(state_pool.tile([128, LB], F16, name=f"x2_{i}"))
        lh2p_t.append(state_pool.tile([128, LB // 2], F16, name=f"lh2p_{i}"))

    def _pad_dmas():
        """Pad/ones constants for tiles not needed in the first block; emitted
        after the block-0 x DMAs so compute starts immediately."""
        for i in range(2):
            t = lh2p_t[i]
            nc.sync.dma_start(t[44:64, :], cons_d[0:20, : LB // 2])
            nc.sync.dma_start(t[108:128, :], cons_d[1:21, : LB // 2])
        for lvl in range(1, 9):
            R = R_LVL[lvl]
            h2p = ST[lvl]["h2p"]
            nc.sync.dma_start(h2p[44:64, :], cons_d[0:20, : R // 2])
            nc.sync.dma_start(h2p[108:128, :], cons_d[1:21, : R // 2])

    # ---- pools
    xpool = ctx.enter_context(tc.tile_pool(name="xpool", bufs=2))
    glpool = ctx.enter_context(tc.tile_pool(name="gl", bufs=3))
    lpool = ctx.enter_context(tc.tile_pool(name="lpool", bufs=2))
    gpool = ctx.enter_context(tc.tile_pool(name="g", bufs=2))
    pspool = ctx.enter_context(tc.tile_pool(name="ps", bufs=4, space="PSUM"))
    tmp1 = ctx.enter_context(tc.tile_pool(name="tmp1", bufs=1))
    tmp2 = ctx.enter_context(tc.tile_pool(name="tmp2", bufs=2))
    tmp3 = ctx.enter_context(tc.tile_pool(name="tmp3", bufs=1))
    opool = ctx.enter_context(tc.tile_pool(name="o", bufs=1))

    # ================================================================ helpers
    def leaf_gemm(xk, s, Gl):
        """Leaf gates for sub-chunk s (512 cols): 4 psum pairs, 5 ACTs."""
        n0 = s * NF
        for pi in range(4):
            ua, ub = 2 * pi, 2 * pi + 1
            ps = pspool.tile([128, 2 * NF], F32, tag="ps", name=f"psl{pi}")
            for j, u in enumerate((ua, ub)):
                rows = (128, 128, 128, 128, 128, 128, 108, 44)[u]
                off = j * NF
                for kc in range(3):
                    nc.tensor.matmul(
                        ps[0:rows, off: off + NF],
                        wleaf_sb[kc][:, 128 * u: 128 * u + rows],
                        xk[kc][:, n0: n0 + NF],
                        start=(kc == 0), stop=(kc == 2))
            if pi < 3:
                func = SIG if pi < 2 else TANH
                nc.scalar.activation(Gl[:, 2 * pi * NF: (2 * pi + 2) * NF],
                                     ps[:, :], func)
            else:
                nc.scalar.activation(Gl[0:108, 6 * NF: 7 * NF],
                                     ps[0:108, 0:NF], SIG)
                nc.scalar.activation(Gl[0:44, 7 * NF: 8 * NF],
                                     ps[0:44, NF: NF + NF], TANH)

    def rec_gemm(rhs_k, PB, G):
        """Recurrent gates for one block of PB cols: 6 psum pairs + 1 single."""
        UROWS = (128,) * 10 + (108, 108, 44)
        for pi in range(7):
            units = (2 * pi, 2 * pi + 1) if pi < 6 else (12,)
            ps = pspool.tile([128, 2 * NF], F32, tag="ps", name=f"psr{pi}")
            for j, u in enumerate(units):
                rows = UROWS[u]
                off = j * PB
                for kc in range(5):
                    nc.tensor.matmul(
                        ps[0:rows, off: off + PB],
                        wrec_sb[kc][:, 128 * u: 128 * u + rows],
                        rhs_k[kc],
                        start=(kc == 0), stop=(kc == 4))
            if pi < 5:
                func = SIG if pi < 4 else TANH
                nc.scalar.activation(G[:, 2 * pi * PB: (2 * pi + 2) * PB],
                                     ps[:, 0: 2 * PB], func)
            elif pi == 5:
                # T10 = [i2@0 | o2@64] all sigmoid; T11 = [u2@0 | fL2@64]
                nc.scalar.activation(G[0:108, 10 * PB: 11 * PB],
                                     ps[0:108, 0:PB], SIG)
                nc.scalar.activation(G[0:44, 11 * PB: 12 * PB],
                                     ps[0:44, PB: 2 * PB], TANH)
                nc.scalar.activation(G[64:108, 11 * PB: 12 * PB],
                                     ps[64:108, PB: 2 * PB], SIG)
            else:
                nc.scalar.activation(G[0:44, 12 * PB: 13 * PB],
                                     ps[0:44, 0:PB], SIG)

    def st_sl(t, R, ch, eo, q0, w):
        off = ch * R + eo * (R // 2) + q0
        return t[:, off: off + w]

    def rec_ew(G, PB, CL, CR, dst, p0):
        """Elementwise for a recurrent block. G gates [128, 13*PB].
        CL/CR: (c0, c1, c2) child-c dense APs [., PB] (c2: 44 rows at base
        64 (CL) / 0 (CR) to match fL2/fR2 partition bases).
        dst: ST[lvl] dict, or ('root', oc01, oc2, oh01, oh2) for level 9."""
        N = PB
        gi = [G[:, 0:N], G[:, N: 2 * N], G[0:44, 10 * N: 11 * N]]
        go = [G[:, 2 * N: 3 * N], G[:, 3 * N: 4 * N], G[64:108, 10 * N: 11 * N]]
        gfL = [G[:, 4 * N: 5 * N], G[:, 5 * N: 6 * N], G[64:108, 11 * N: 12 * N]]
        gfR = [G[:, 6 * N: 7 * N], G[:, 7 * N: 8 * N], G[0:44, 12 * N: 13 * N]]
        gu = [G[:, 8 * N: 9 * N], G[:, 9 * N: 10 * N], G[0:44, 11 * N: 12 * N]]

        t1 = tmp1.tile([128, 2 * NF], F16, tag="t1", name="t1")
        t2 = tmp1.tile([128, 2 * NF], F16, tag="t2", name="t2")
        fc = tmp1.tile([128, 2 * NF], F16, tag="fc", name="fc")
        iu = tmp1.tile([128, 2 * NF], F16, tag="iu", name="iu")
        t1_2 = tmp1.tile([44, NF], F16, tag="t1_2", name="t1_2")
        t2_2 = tmp1.tile([44, NF], F16, tag="t2_2", name="t2_2")
        fc2 = tmp1.tile([44, NF], F16, tag="fc2", name="fc2")
        iu2 = tmp1.tile([44, NF], F16, tag="iu2", name="iu2")

        for ch in range(2):
            nc.vector.tensor_tensor(t1[:, ch * N: (ch + 1) * N], gfL[ch],
                                    CL[ch], MUL)
            nc.vector.tensor_tensor(t2[:, ch * N: (ch + 1) * N], gfR[ch],
                                    CR[ch], MUL)
        nc.vector.tensor_tensor(t1_2[:, :N], gfL[2], CL[2], MUL)
        nc.vector.tensor_tensor(t2_2[:, :N], gfR[2], CR[2], MUL)
        nc.vector.tensor_tensor(fc[:, : 2 * N], t1[:, : 2 * N], t2[:, : 2 * N],
                                ADD)
        nc.vector.tensor_tensor(fc2[:, :N], t1_2[:, :N], t2_2[:, :N], ADD)
        nc.vector.tensor_tensor(iu[:, : 2 * N], G[:, 0: 2 * N],
                                G[:, 8 * N: 10 * N], MUL)
        nc.vector.tensor_tensor(iu2[:, :N], gi[2], gu[2], MUL)

        if isinstance(dst, tuple) and dst[0] == "root":
            _, oc01, oc2, oh01, oh2 = dst
            nc.vector.tensor_tensor(oc01[:, : 2 * N], iu[:, : 2 * N],
                                    fc[:, : 2 * N], ADD)
            nc.vector.tensor_tensor(oc2[:, :N], iu2[:, :N], fc2[:, :N], ADD)
            th = tmp2.tile([128, 2 * NF], F16, tag="th", name="th")
            th2 = tmp3.tile([128, NF], F16, tag="th2", name="th2")
            nc.scalar.activation(th[:, : 2 * N], oc01[:, : 2 * N], TANH)
            nc.scalar.activation(th2[64:108, :N], oc2[:, :N], TANH)
            nc.vector.tensor_tensor(oh01[:, : 2 * N], G[:, 2 * N: 4 * N],
                                    th[:, : 2 * N], MUL)
            nc.vector.tensor_tensor(oh2[:, :N], go[2], th2[64:108, :N], MUL)
            return

        st = dst
        R = st["R"]
        q0, hw = p0 // 2, PB // 2
        # c writes (deinterleave into state), then tanh, then h writes
        for ch in range(2):
            iue, iuo = _ev_od(iu[:, ch * N: (ch + 1) * N])
            fce, fco = _ev_od(fc[:, ch * N: (ch + 1) * N])
            nc.vector.tensor_tensor(_mb(st_sl(st["c01"], R, ch, 0, q0, hw)),
                                    iue, fce, ADD)
            nc.vector.tensor_tensor(_mb(st_sl(st["c01"], R, ch, 1, q0, hw)),
                                    iuo, fco, ADD)
        iue, iuo = _ev_od(iu2[:, :N])
        fce, fco = _ev_od(fc2[:, :N])
        nc.vector.tensor_tensor(_mb(st["c2p"][64:108, q0: q0 + hw]), iue, fce,
                                ADD)
        nc.vector.tensor_tensor(_mb(st["c2p"][0:44, q0: q0 + hw]), iuo, fco,
                                ADD)

        # th layout: [ch0E | ch0O | ch1E | ch1O], each hw wide
        th = tmp2.tile([128, 2 * NF], F16, tag="th", name="th")
        th2 = tmp3.tile([128, NF], F16, tag="th2", name="th2")
        c4 = st["c01"].rearrange("p (ch eo q) -> p ch eo q", ch=2, eo=2)
        tho = th[:, : 2 * N].rearrange("p (ch eo q) -> p ch eo q", ch=2, eo=2)
        nc.scalar.activation(tho, c4[:, :, :, q0: q0 + hw], TANH)
        nc.scalar.activation(th2[64:108, 0:hw], st["c2p"][64:108, q0: q0 + hw],
                             TANH)
        nc.scalar.activation(th2[64:108, hw:N], st["c2p"][0:44, q0: q0 + hw],
                             TANH)

        for ch in range(2):
            oe, oo = _ev_od(go[ch])
            nc.vector.tensor_tensor(_mb(st_sl(st["h01"], R, ch, 0, q0, hw)),
                                    oe, _mb(th[:, ch * N: ch * N + hw]), MUL)
            nc.vector.tensor_tensor(
                _mb(st_sl(st["h01"], R, ch, 1, q0, hw)), oo,
                _mb(th[:, ch * N + hw: ch * N + N]), MUL)
        oe, oo = _ev_od(go[2])
        nc.vector.tensor_tensor(_mb(st["h2p"][0:44, q0: q0 + hw]), oe,
                                _mb(th2[64:108, 0:hw]), MUL)
        nc.vector.tensor_tensor(_mb(st["h2p"][64:108, q0: q0 + hw]), oo,
                                _mb(th2[64:108, hw:N]), MUL)

    def leaf_ew(Gl, s, lh01, lh2p, lc01, lc2p):
        """Leaf elementwise for sub-chunk s (512 cols): c = i*u, h = o*tanh(c).
        Writes deinterleaved into the LB-wide block-local leaf tiles."""
        N = NF
        q0, hw = s * (NF // 2), NF // 2
        gi = [Gl[:, 0:N], Gl[:, N: 2 * N], Gl[0:44, 6 * N: 7 * N]]
        go = [Gl[:, 2 * N: 3 * N], Gl[:, 3 * N: 4 * N], Gl[64:108, 6 * N: 7 * N]]
        gu = [Gl[:, 4 * N: 5 * N], Gl[:, 5 * N: 6 * N], Gl[0:44, 7 * N: 8 * N]]

        for ch in range(2):
            ie, io = _ev_od(gi[ch])
            ue, uo = _ev_od(gu[ch])
            nc.vector.tensor_tensor(_mb(st_sl(lc01, LB, ch, 0, q0, hw)), ie, ue,
                                    MUL)
            nc.vector.tensor_tensor(_mb(st_sl(lc01, LB, ch, 1, q0, hw)), io, uo,
                                    MUL)
        i2e, i2o = _ev_od(gi[2])
        u2e, u2o = _ev_od(gu[2])
        nc.vector.tensor_tensor(_mb(lc2p[64:108, q0: q0 + hw]), i2e, u2e, MUL)
        nc.vector.tensor_tensor(_mb(lc2p[0:44, q0: q0 + hw]), i2o, u2o, MUL)

        th = tmp2.tile([128, 2 * NF], F16, tag="thl", name="lth")
        th2 = tmp3.tile([128, NF], F16, tag="th2l", name="lth2")
        c4 = lc01.rearrange("p (ch eo q) -> p ch eo q", ch=2, eo=2)
        tho = th[:, : 2 * N].rearrange("p (ch eo q) -> p ch eo q", ch=2, eo=2)
        nc.scalar.activation(tho, c4[:, :, :, q0: q0 + hw], TANH)
        nc.scalar.activation(th2[64:108, 0:hw], lc2p[64:108, q0: q0 + hw],
                             TANH)
        nc.scalar.activation(th2[64:108, hw:N], lc2p[0:44, q0: q0 + hw], TANH)

        for ch in range(2):
            oe, oo = _ev_od(go[ch])
            nc.vector.tensor_tensor(_mb(st_sl(lh01, LB, ch, 0, q0, hw)), oe,
                                    _mb(th[:, ch * N: ch * N + hw]), MUL)
            nc.vector.tensor_tensor(
                _mb(st_sl(lh01, LB, ch, 1, q0, hw)), oo,
                _mb(th[:, ch * N + hw: ch * N + N]), MUL)
        oe, oo = _ev_od(go[2])
        nc.vector.tensor_tensor(_mb(lh2p[0:44, q0: q0 + hw]), oe,
                                _mb(th2[64:108, 0:hw]), MUL)
        nc.vector.tensor_tensor(_mb(lh2p[64:108, q0: q0 + hw]), oo,
                                _mb(th2[64:108, hw:N]), MUL)

    # ================================================================ phase A
    # leaves + level-1, software-pipelined: L1 GEMM of block b-1 is emitted
    # after the leaf GEMMs of block b so the PE never waits on leaf DVE.
    n_blk = XCOLS // LB                       # 8 blocks
    pend = None                               # (lh01, lh2p, lc01, lc2p, blk)

    def l1_block(lh01, lh2p, lc01, lc2p, blk):
        rhs_k = [st_sl(lh01, LB, 0, 0, 0, NF), st_sl(lh01, LB, 1, 0, 0, NF),
                 lh2p[:, :],
                 st_sl(lh01, LB, 0, 1, 0, NF), st_sl(lh01, LB, 1, 1, 0, NF)]
        G = gpool.tile([128, 13 * NF], F16, tag="G", name="G1")
        rec_gemm(rhs_k, NF, G)
        CL = [st_sl(lc01, LB, 0, 0, 0, NF), st_sl(lc01, LB, 1, 0, 0, NF),
              lc2p[64:108, :]]
        CR = [st_sl(lc01, LB, 0, 1, 0, NF), st_sl(lc01, LB, 1, 1, 0, NF),
              lc2p[0:44, :]]
        rec_ew(G, NF, CL, CR, ST[1], blk * NF)

    for blk in range(n_blk):
        c0 = blk * LB
        x01 = xpool.tile([128, 2 * LB], F16, tag="x01", name="x01")
        x2 = x2_t[blk % 2]
        nc.sync.dma_start(x01.rearrange("p (two b) -> p two b", two=2),
                          xt_d[:, :, c0: c0 + LB])
        nc.sync.dma_start(x2[0:44, :], x2t_d[:, c0: c0 + LB])
        if blk == 0:
            nc.sync.dma_start(x2[44:128, :], cons_d[0:84, :LB])
            nc.sync.dma_start(wrec_t[:], wrec_d[:])
            nc.sync.dma_start(x2_t[1][44:128, :], cons_d[0:84, :LB])
        elif blk == 1:
            _pad_dmas()
        xk = [x01[:, 0:LB], x01[:, LB: 2 * LB], x2]

        lh01 = lpool.tile([128, 2 * LB], F16, tag="lh01", name="lh01")
        lh2p = lh2p_t[blk % 2]
        lc01 = lpool.tile([128, 2 * LB], F16, tag="lc01", name="lc01")
        lc2p = lpool.tile([128, LB // 2], F16, tag="lc2p", name="lc2p")

        Gls = []
        for s in range(2):
            Gl = glpool.tile([128, 8 * NF], F16, tag="Gl", name="Gl")
            leaf_gemm(xk, s, Gl)
            Gls.append(Gl)
        if pend is not None:
            l1_block(*pend)
        for s in range(2):
            leaf_ew(Gls[s], s, lh01, lh2p, lc01, lc2p)
        pend = (lh01, lh2p, lc01, lc2p, blk)
    l1_block(*pend)

    # ================================================================ phase B
    for lvl in range(2, 10):
        R = R_LVL[lvl]
        Rp = R_LVL[lvl - 1]
        PB = min(NF, R)
        prev = ST[lvl - 1]
        for p0 in range(0, R, PB):
            rhs_k = [st_sl(prev["h01"], Rp, 0, 0, p0, PB),
                     st_sl(prev["h01"], Rp, 1, 0, p0, PB),
                     prev["h2p"][:, p0: p0 + PB],
                     st_sl(prev["h01"], Rp, 0, 1, p0, PB),
                     st_sl(prev["h01"], Rp, 1, 1, p0, PB)]
            G = gpool.tile([128, 13 * NF], F16, tag="G", name=f"G{lvl}")
            rec_gemm(rhs_k, PB, G[:, : 13 * PB])
            CL = [st_sl(prev["c01"], Rp, 0, 0, p0, PB),
                  st_sl(prev["c01"], Rp, 1, 0, p0, PB),
                  prev["c2p"][64:108, p0: p0 + PB]]
            CR = [st_sl(prev["c01"], Rp, 0, 1, p0, PB),
                  st_sl(prev["c01"], Rp, 1, 1, p0, PB),
                  prev["c2p"][0:44, p0: p0 + PB]]
            if lvl < 9:
                rec_ew(G[:, : 13 * PB], PB, CL, CR, ST[lvl], p0)
            else:
                oc01 = opool.tile([128, 2 * B_LOC], F16, name="oc01")
                oc2 = opool.tile([44, B_LOC], F16, name="oc2")
                oh01 = opool.tile([128, 2 * B_LOC], F16, name="oh01")
                oh2 = opool.tile([44, B_LOC], F16, name="oh2")
                rec_ew(G[:, : 13 * PB], PB, CL, CR,
                       ("root", oc01, oc2, oh01, oh2), p0)
                nc.sync.dma_start(out_d[0, 0:128, :], oc01[:, 0:B_LOC])
                nc.sync.dma_start(out_d[0, 128:256, :], oc01[:, B_LOC: 2 * B_LOC])
                nc.sync.dma_start(out_d[0, 256:300, :], oc2[:, :])
                nc.sync.dma_start(out_d[1, 0:128, :], oh01[:, 0:B_LOC])
                nc.sync.dma_start(out_d[1, 128:256, :], oh01[:, B_LOC: 2 * B_LOC])
                nc.sync.dma_start(out_d[1, 256:300, :], oh2[:, :])


# ---------------------------------------------------------------- runner
_CACHE = {}


def _get_program():
    if "nc" not in _CACHE:
        _CACHE["nc"] = build_program()
    return _CACHE["nc"]


def _host_inputs(inputs, Wfioux, b_fioux, Wiouh, Wfh):
    wleaf, wrec = _pack_weights(
        np.asarray(Wfioux, np.float32), np.asarray(b_fioux, np.float32),
        np.asarray(Wiouh, np.float32), np.asarray(Wfh, np.float32))
    cons = np.zeros((84, 2 * LB), np.float16)
    cons[0, :] = 1.0
    in_maps = []
    for core in range(N_CORES):
        x = np.asarray(inputs[core * B_LOC:(core + 1) * B_LOC, :N_LEAVES, :],
                       np.float32)
        xt_full = x.transpose(2, 1, 0).reshape(MEM, XCOLS).astype(np.float16)
        xt = np.ascontiguousarray(xt_full[0:256].reshape(2, 128, XCOLS)
                                  .transpose(1, 0, 2))
        x2t = np.ascontiguousarray(xt_full[256:300])
        in_maps.append({"xt": xt, "x2t": x2t, "wleaf": wleaf, "wrec": wrec,
                        "cons": cons})
    return in_maps


def kernel(inputs, Wfioux, b_fioux, Wiouh, Wfh, left_idx, right_idx, leaf_mask,
           _trace=False, _trace_dir=None):
    inputs = np.asarray(inputs, np.float32)
    assert _check_topology(left_idx, right_idx, leaf_mask), \
        "tree topology does not match the expected complete binary tree"

    in_maps = _host_inputs(inputs, Wfioux, b_fioux, Wiouh, Wfh)
    nc = _get_program()
    res = run_bass_kernel_spmd(nc, in_maps, list(range(N_CORES)),
                               trace=_trace, tmpdir=_trace_dir)

    root_c = np.zeros((B, MEM), np.float32)
    root_h = np.zeros((B, MEM), np.float32)
    for core in range(N_CORES):
        out = np.asarray(res.results[core]["out"], np.float32)  # [2, 300, 16]
        root_c[core * B_LOC:(core + 1) * B_LOC] = out[0].T
        root_h[core * B_LOC:(core + 1) * B_LOC] = out[1].T
    _CACHE["last_results"] = res
    return root_c, root_h
